# revision 13
# baseline (speedup 1.0000x reference)
"""Trainium2 Bass kernel for nn_Basis (gaussian-basis orbital evaluation).

out[i, m] = sum_{p: orbital_index[p]==m} coeff[p]*norm[p]
            * prod_c (pos[i,c]-center[p,c])^lmn[p,c] * exp(-alpha[p]*|pos_i-center_p|^2)

Strategy (8 NeuronCores, data-parallel over points):
  - Host: Morton-sort points into 512-point windows with local origins.
    mono/expo expanded in dp = pos - o features as bf16-limb polynomials
    (81 / 30 K-rows, zero-padded to K=128: narrow-K matmuls hit a HW
    double-accumulate hazard; K=128 also enables FWL + 1 cyc/col).
  - Sparsity: per (core, window) only prims whose peak contribution over
    the window exceeds TOL are kept (~42%). Active prims are packed into
    128-wide chunk slots, split by orbital half (tile0: orb<128, tile1:
    orb>=128) so each chunk's segment matrix targets one PSUM half.
    A fixed per-rank capacity profile (max over cores) keeps the program
    identical across cores; windows are assigned to slots by rank.
  - Device per chunk slot:
      PE:  mono = Bmono^T @ A, expo = Bexpo^T @ Ae   (bf16, K=128)
      ACT: e = exp(expo) -> bf16
      DVE: prim = mono * e -> bf16
      PE:  pot[tile-half] += S_chunk^T @ prim   (0/1 bf16, PSUM accum)
  - Per-window [128, 2*512] PSUM out tile copied bf16 (ACT/DVE alternate)
    and DMA'd; host inverts slot assignment, Morton perm, orbital tiling.
"""
import os
import sys

sys.path.insert(0, "/opt/trn_rl_repo")

import numpy as np

import concourse.bass as bass
from concourse import bacc, mybir, tile
from concourse._compat import with_exitstack  # noqa: F401

import ml_dtypes

BF16 = mybir.dt.bfloat16
F32 = mybir.dt.float32
AF = mybir.ActivationFunctionType
NP_BF16 = ml_dtypes.bfloat16

N_POINTS = 65536
N_PRIM = 1024
N_ORB = 256
N_CORES = 8
N_SH = N_POINTS // N_CORES  # 8192 points per core
WIN = 512                   # points per window
PCH = 128                   # prims per chunk slot
N_WIN = N_SH // WIN         # 16 windows per core
KM = 128                    # K rows (81 mono / 30 expo used, rest zero)
TOL = 5e-4                  # abs prim-contribution cutoff

_EXPS = [(a, b, c) for a in range(3) for b in range(3) for c in range(3)]
_BINOM = np.array([[1, 0, 0], [1, 1, 0], [1, 2, 1]], dtype=np.float64)


def _morton_perm(pos):
    n = pos.shape[0]
    q = np.empty((n, 3), np.uint64)
    for d in range(3):
        x = pos[:, d].astype(np.float64)
        lo, hi = x.min(), x.max()
        q[:, d] = np.clip((x - lo) / max(hi - lo, 1e-9) * 1023.0, 0, 1023).astype(
            np.uint64
        )
    code = np.zeros(n, np.uint64)
    for b in range(10):
        for d in range(3):
            code |= ((q[:, d] >> np.uint64(b)) & np.uint64(1)) << np.uint64(3 * b + d)
    return np.argsort(code, kind="stable")


def _limbs(x, n):
    """Split f64 array into n bf16 limbs: x ~= sum(limbs)."""
    out = []
    r = x.copy()
    for _ in range(n):
        h = r.astype(NP_BF16)
        out.append(h)
        r = r - h.astype(np.float64)
    return out


def _host_prep(pos, coefficients, norm, center, alpha, lmn, orbital_index):
    """Returns (per_core in_maps, perm, profile (T0,T1 tuples), per-core
    window order arrays)."""
    pos = np.asarray(pos, np.float64)
    cn = (np.asarray(coefficients, np.float64) * np.asarray(norm, np.float64))
    center = np.asarray(center, np.float64)
    alpha = np.asarray(alpha, np.float64)
    lmn = np.asarray(lmn, np.int64)
    seg = np.asarray(orbital_index, np.int64)

    perm = _morton_perm(pos)
    spos = pos[perm]
    wpos = spos.reshape(N_CORES, N_WIN, WIN, 3)

    # ---- active prims per (core, window) ----
    sub = wpos[:, :, ::8, :]  # 64 sample points
    active = np.zeros((N_CORES, N_WIN, N_PRIM), bool)
    for k in range(N_CORES):
        for w in range(N_WIN):
            dx = sub[k, w][None, :, :] - center[:, None, :]
            r2 = (dx * dx).sum(-1)
            mono = (np.abs(dx) ** lmn[:, None, :]).prod(-1)
            v = np.abs(cn)[:, None] * mono * np.exp(-alpha[:, None] * r2)
            active[k, w] = v.max(1) > TOL

    in0 = seg < 128
    n0 = (active & in0[None, None, :]).sum(-1)
    n1 = (active & ~in0[None, None, :]).sum(-1)
    t0 = np.maximum(-(-n0 // PCH), 1)  # [K, W] ceil, min 1
    t1 = np.maximum(-(-n1 // PCH), 1)
    tot = t0 + t1
    order = np.argsort(-tot, axis=1, kind="stable")  # per-core rank -> window
    t0s = np.take_along_axis(t0, order, 1)
    t1s = np.take_along_axis(t1, order, 1)
    T0 = tuple(int(x) for x in t0s.max(0))
    T1 = tuple(int(x) for x in t1s.max(0))
    ch_of_slot = np.cumsum([0] + [T0[i] + T1[i] for i in range(N_WIN)])
    tot_ch = int(ch_of_slot[-1])

    ln2 = float(np.log(2.0))
    in_maps = []
    for k in range(N_CORES):
        cpos = spos[k * N_SH:(k + 1) * N_SH]
        blocks = cpos.reshape(N_WIN, WIN, 3)
        origins = blocks.mean(axis=1)
        dp0 = blocks - origins[:, None, :]
        lam = np.exp2(
            np.ceil(np.log2(np.maximum(np.abs(dp0).max(axis=(1, 2)), 1e-6) / 4.0))
        ).clip(min=1.0)
        dp = (dp0 / lam[:, None, None]).reshape(N_SH, 3)

        # A features (window-major), then permuted to slot order
        dpow = np.empty((3, 3, N_SH), np.float64)
        for d in range(3):
            dpow[d, 0] = 1.0
            dpow[d, 1] = dp[:, d]
            dpow[d, 2] = dp[:, d] ** 2
        a_mono = np.empty((27, N_SH), np.float64)
        for ki, (a, b, c) in enumerate(_EXPS):
            a_mono[ki] = dpow[0, a] * dpow[1, b] * dpow[2, c]
        r2p = dp[:, 0] ** 2 + dp[:, 1] ** 2 + dp[:, 2] ** 2
        a_expo = np.stack(
            [np.ones(N_SH), dp[:, 0], dp[:, 1], dp[:, 2], r2p], axis=0
        )
        am0, am1 = _limbs(a_mono, 2)
        at_m = np.zeros((KM, N_SH), NP_BF16)
        at_m[:81] = np.concatenate([am0, am1, am0], axis=0)
        ae0, ae1, ae2 = _limbs(a_expo, 3)
        at_e = np.zeros((KM, N_SH), NP_BF16)
        at_e[:30] = np.concatenate([ae0, ae1, ae2, ae0, ae1, ae0], axis=0)
        ord_k = order[k]
        at_m = at_m.reshape(KM, N_WIN, WIN)[:, ord_k, :].reshape(KM, N_SH)
        at_e = at_e.reshape(KM, N_WIN, WIN)[:, ord_k, :].reshape(KM, N_SH)

        # B tables per (window, prim), window-major
        cpr = center[None, :, :] - origins[:, None, :]  # [W, P, 3]
        npow = np.empty((N_WIN, N_PRIM, 3, 3), np.float64)
        npow[..., 0] = 1.0
        npow[..., 1] = -cpr
        npow[..., 2] = cpr ** 2
        bc = np.empty((N_WIN, N_PRIM, 3, 3), np.float64)
        for d in range(3):
            ld = lmn[:, d]
            for e in range(3):
                valid = (e <= ld)
                bcoef = _BINOM[ld, e]
                pw = npow[:, np.arange(N_PRIM), d, ld - e]
                bc[:, :, d, e] = np.where(valid[None, :], bcoef[None, :] * pw, 0.0)
        coefm = np.empty((N_WIN, N_PRIM, 27), np.float64)
        for ki, (a, b, c) in enumerate(_EXPS):
            coefm[:, :, ki] = (
                bc[:, :, 0, a] * bc[:, :, 1, b] * bc[:, :, 2, c]
                * (lam[:, None] ** (a + b + c))
            )
        coefm *= cn[None, :, None]
        maxc = np.abs(coefm).max(axis=2)
        s = np.ceil(np.log2(np.maximum(maxc, 1e-300) / 30000.0)).clip(min=0.0)
        coefm *= 2.0 ** (-s[:, :, None])

        c2 = (cpr ** 2).sum(axis=2)
        coefe = np.empty((N_WIN, N_PRIM, 5), np.float64)
        coefe[:, :, 0] = -alpha[None, :] * c2 + s * ln2
        for d in range(3):
            coefe[:, :, 1 + d] = 2.0 * alpha[None, :] * cpr[:, :, d] * lam[:, None]
        coefe[:, :, 4] = -alpha[None, :] * (lam ** 2)[:, None]

        bm0, bm1 = _limbs(coefm.transpose(0, 2, 1), 2)  # [W, 27, P]
        b_m = np.zeros((N_WIN, KM, N_PRIM), NP_BF16)
        b_m[:, :81] = np.concatenate([bm0, bm0, bm1], axis=1)
        be0, be1, be2 = _limbs(coefe.transpose(0, 2, 1), 3)
        b_e = np.zeros((N_WIN, KM, N_PRIM), NP_BF16)
        b_e[:, :30] = np.concatenate([be0, be0, be0, be1, be1, be2], axis=1)

        # ---- pack active prims into chunk slots ----
        # prims_of_chunk[ch, lane] = global prim id or -1
        prims = np.full((tot_ch, PCH), -1, np.int64)
        tile_of_chunk = np.zeros(tot_ch, np.int64)
        win_of_chunk = np.zeros(tot_ch, np.int64)
        for i in range(N_WIN):
            w = ord_k[i]
            act = active[k, w]
            p0 = np.nonzero(act & in0)[0]
            p1 = np.nonzero(act & ~in0)[0]
            base = ch_of_slot[i]
            for j in range(T0[i]):
                sel = p0[j * PCH:(j + 1) * PCH]
                prims[base + j, :len(sel)] = sel
                tile_of_chunk[base + j] = 0
                win_of_chunk[base + j] = w
            for j in range(T1[i]):
                ch = base + T0[i] + j
                sel = p1[j * PCH:(j + 1) * PCH]
                prims[ch, :len(sel)] = sel
                tile_of_chunk[ch] = 1
                win_of_chunk[ch] = w

        valid = prims >= 0
        pidx = np.where(valid, prims, 0)
        bm_pk = b_m[win_of_chunk[:, None], :, pidx]   # [CH, PCH, KM]
        be_pk = b_e[win_of_chunk[:, None], :, pidx]
        bm_pk[~valid] = 0
        be_pk[~valid] = 0
        bm_pk = np.ascontiguousarray(
            bm_pk.transpose(2, 0, 1).reshape(KM, tot_ch * PCH))
        be_pk = np.ascontiguousarray(
            be_pk.transpose(2, 0, 1).reshape(KM, tot_ch * PCH))

        s_pk = np.zeros((PCH, tot_ch * PCH), NP_BF16)
        ch_i, lane_i = np.nonzero(valid)
        orb = seg[prims[ch_i, lane_i]] - 128 * tile_of_chunk[ch_i]
        s_pk[lane_i, ch_i * PCH + orb] = 1.0

        in_maps.append(
            {
                "am": np.ascontiguousarray(at_m),
                "ae": np.ascontiguousarray(at_e),
                "bm": bm_pk,
                "be": be_pk,
                "s_pk": s_pk,
            }
        )
    return in_maps, perm, (T0, T1), order


def build_program(profile, n_sh=N_SH):
    T0, T1 = profile
    ch_of_slot = np.cumsum([0] + [T0[i] + T1[i] for i in range(N_WIN)])
    tot_ch = int(ch_of_slot[-1])
    nc = bacc.Bacc("TRN2", target_bir_lowering=False, debug=False,
                   num_devices=N_CORES)
    am_d = nc.dram_tensor("am", [KM, n_sh], BF16, kind="ExternalInput").ap()
    ae_d = nc.dram_tensor("ae", [KM, n_sh], BF16, kind="ExternalInput").ap()
    bm_d = nc.dram_tensor("bm", [KM, tot_ch * PCH], BF16, kind="ExternalInput").ap()
    be_d = nc.dram_tensor("be", [KM, tot_ch * PCH], BF16, kind="ExternalInput").ap()
    s_pk_d = nc.dram_tensor("s_pk", [PCH, tot_ch * PCH], BF16,
                            kind="ExternalInput").ap()
    out_d = nc.dram_tensor("out_t", [128, 2, n_sh], BF16, kind="ExternalOutput").ap()

    with tile.TileContext(nc) as tc:
        with (
            tc.tile_pool(name="cst", bufs=1) as cst,
            tc.tile_pool(name="wk", bufs=8) as wk,
            tc.tile_pool(name="ob", bufs=4) as ob,
            tc.tile_pool(name="pm", bufs=3, space="PSUM") as pm,
            tc.tile_pool(name="pex", bufs=3, space="PSUM") as pex,
            tc.tile_pool(name="po0", bufs=1, space="PSUM") as po0,
            tc.tile_pool(name="po1", bufs=1, space="PSUM") as po1,
        ):
            am_t = cst.tile([KM, n_sh], BF16)
            ae_t = cst.tile([KM, n_sh], BF16)
            bm_t = cst.tile([KM, tot_ch * PCH], BF16)
            be_t = cst.tile([KM, tot_ch * PCH], BF16)
            s_t = cst.tile([PCH, tot_ch * PCH], BF16)
            nsplit = 4
            for i in range(nsplit):
                wsl = slice(i * (n_sh // nsplit), (i + 1) * (n_sh // nsplit))
                c0 = (i * tot_ch // nsplit) * PCH
                c1 = ((i + 1) * tot_ch // nsplit) * PCH
                nc.sync.dma_start(am_t[:, wsl], am_d[:, wsl])
                nc.sync.dma_start(ae_t[:, wsl], ae_d[:, wsl])
                nc.sync.dma_start(bm_t[:, c0:c1], bm_d[:, c0:c1])
                nc.sync.dma_start(be_t[:, c0:c1], be_d[:, c0:c1])
                nc.sync.dma_start(s_t[:, c0:c1], s_pk_d[:, c0:c1])

            for i in range(N_WIN):
                pot0 = po0.tile([128, WIN], F32, tag="outp0")
                pot1 = po1.tile([128, WIN], F32, tag="outp1")
                pot = [pot0, pot1]
                psl = slice(i * WIN, (i + 1) * WIN)
                base = int(ch_of_slot[i])
                nch = T0[i] + T1[i]
                for j in range(nch):
                    ch = base + j
                    tl = 0 if j < T0[i] else 1
                    jj = j if tl == 0 else j - T0[i]
                    lastj = (T0[i] - 1) if tl == 0 else (nch - T0[i] - 1)
                    bsl = slice(ch * PCH, (ch + 1) * PCH)
                    mono_p = pm.tile([128, WIN], F32, tag="mono")
                    expo_p = pex.tile([128, WIN], F32, tag="expo")
                    nc.tensor.matmul(
                        mono_p[:], bm_t[:, bsl], am_t[:, psl],
                        start=True, stop=True,
                    )
                    nc.tensor.matmul(
                        expo_p[:], be_t[:, bsl], ae_t[:, psl],
                        start=True, stop=True,
                    )
                    e_t = wk.tile([128, WIN], BF16, tag="e")
                    nc.scalar.activation(e_t[:], expo_p[:], AF.Exp)
                    prim_t = wk.tile([128, WIN], BF16, tag="prim")
                    nc.vector.tensor_mul(prim_t[:], mono_p[:], e_t[:])
                    nc.tensor.matmul(
                        pot[tl][:],
                        s_t[:, bsl],
                        prim_t[:],
                        start=(jj == 0),
                        stop=(jj == lastj),
                    )
                # parallel PSUM->SBUF drains: ACT takes one half, DVE the
                # other (alternating), so the window boundary stalls on a
                # single 512-col copy instead of a serial 1024-col one
                osb = ob.tile([128, 2 * WIN], BF16, tag="osb")
                nc.scalar.copy(osb[:, (i % 2) * WIN:(i % 2 + 1) * WIN],
                               pot[i % 2][:])
                nc.vector.tensor_copy(
                    osb[:, (1 - i % 2) * WIN:(2 - i % 2) * WIN],
                    pot[1 - i % 2][:])
                for t in range(2):
                    nc.sync.dma_start(
                        out_d[:, t, psl], osb[:, t * WIN:(t + 1) * WIN]
                    )
    nc.compile()
    return nc


_PROG_CACHE = {}


def _get_program(profile):
    if profile not in _PROG_CACHE:
        _PROG_CACHE[profile] = build_program(profile)
    return _PROG_CACHE[profile]


def _install_ntff_hook_shim():
    """The agent image's antenv lacks axon_hooks; synthesize it so
    run_bass_kernel_spmd(trace=True) can capture NTFF profiles."""
    try:
        from antenv.axon_hooks import get_axon_ntff_profile_hook  # noqa: F401
        return True
    except ImportError:
        pass
    try:
        import types
        import antenv
        from trn_agent_boot.trn_boot import _ntff_profile_via_ctypes

        hook = _ntff_profile_via_ctypes("/opt/axon/libaxon_pjrt.so")
        mod = types.ModuleType("antenv.axon_hooks")
        mod._hook = hook
        mod.set_axon_ntff_profile_hook = lambda h: setattr(mod, "_hook", h)
        mod.get_axon_ntff_profile_hook = lambda: mod._hook
        sys.modules["antenv.axon_hooks"] = mod
        antenv.axon_hooks = mod
        return True
    except Exception as e:  # pragma: no cover
        print(f"ntff hook shim failed ({e}); running without trace")
        return False


def kernel(pos, coefficients, norm, center, alpha, lmn, orbital_index,
           num_orbitals):
    assert int(num_orbitals) == N_ORB and pos.shape == (N_POINTS, 3)
    in_maps, perm, profile, order = _host_prep(
        pos, coefficients, norm, center, alpha, lmn, orbital_index
    )
    nc = _get_program(profile)

    from concourse.bass_utils import run_bass_kernel_spmd

    trace = bool(os.environ.get("BASS_KERNEL_TRACE"))
    if trace:
        trace = _install_ntff_hook_shim()
    res = run_bass_kernel_spmd(nc, in_maps, list(range(N_CORES)), trace=trace)
    kernel.last_results = res

    full = np.empty((N_POINTS, N_ORB), np.float32)
    for k in range(N_CORES):
        v = res.results[k]["out_t"]  # [128, 2, N_SH] in slot order
        orb = v.transpose(1, 0, 2).reshape(N_ORB, N_WIN, WIN)
        orb = orb[:, np.argsort(order[k]), :].reshape(N_ORB, N_SH)
        full[k * N_SH:(k + 1) * N_SH] = orb.T.astype(np.float32)
    out = np.empty_like(full)
    out[perm] = full
    return out


# revision 17
# speedup vs baseline: 1.1441x; 1.1441x over previous
"""Trainium2 Bass kernel for nn_Basis (gaussian-basis orbital evaluation).

out[i, m] = sum_{p: orbital_index[p]==m} coeff[p]*norm[p]
            * prod_c (pos[i,c]-center[p,c])^lmn[p,c] * exp(-alpha[p]*|pos_i-center_p|^2)

Strategy (8 NeuronCores, data-parallel over points):
  - Host: Morton-sort points into 512-point windows with local origins.
    mono/expo expanded in dp = pos - o features as bf16-limb polynomials
    (81 / 30 K-rows, zero-padded to K=128: narrow-K matmuls hit a HW
    double-accumulate hazard; K=128 also enables FWL + 1 cyc/col).
  - Sparsity: per (core, window) only prims whose peak contribution over
    the window exceeds TOL are kept (~42%). Active prims are packed into
    128-wide chunk slots, split by orbital half (tile0: orb<128, tile1:
    orb>=128) so each chunk's segment matrix targets one PSUM half.
    A fixed per-rank capacity profile (max over cores) keeps the program
    identical across cores; windows are assigned to slots by rank.
  - Device per chunk slot:
      PE:  mono = Bmono^T @ A, expo = Bexpo^T @ Ae   (bf16, K=128)
      ACT: e = exp(expo) -> bf16
      DVE: prim = mono * e -> bf16
      PE:  pot[tile-half] += S_chunk^T @ prim   (0/1 bf16, PSUM accum)
  - Per-window [128, 2*512] PSUM out tile copied bf16 (ACT/DVE alternate)
    and DMA'd; host inverts slot assignment, Morton perm, orbital tiling.
"""
import os
import sys

sys.path.insert(0, "/opt/trn_rl_repo")

import numpy as np

import concourse.bass as bass
from concourse import bacc, mybir, tile
from concourse._compat import with_exitstack  # noqa: F401

import ml_dtypes

BF16 = mybir.dt.bfloat16
F32 = mybir.dt.float32
AF = mybir.ActivationFunctionType
NP_BF16 = ml_dtypes.bfloat16

N_POINTS = 65536
N_PRIM = 1024
N_ORB = 256
N_CORES = 8
N_SH = N_POINTS // N_CORES  # 8192 points per core
WIN = 512                   # points per window
PCH = 128                   # prims per chunk slot
N_WIN = N_SH // WIN         # 16 windows per core
KM = 128                    # K rows (81 mono / 30 expo used, rest zero)
TOL = 5e-4                  # abs prim-contribution cutoff

_EXPS = [(a, b, c) for a in range(3) for b in range(3) for c in range(3)]
_BINOM = np.array([[1, 0, 0], [1, 1, 0], [1, 2, 1]], dtype=np.float64)


def _morton_perm(pos):
    n = pos.shape[0]
    q = np.empty((n, 3), np.uint64)
    for d in range(3):
        x = pos[:, d].astype(np.float64)
        lo, hi = x.min(), x.max()
        q[:, d] = np.clip((x - lo) / max(hi - lo, 1e-9) * 1023.0, 0, 1023).astype(
            np.uint64
        )
    code = np.zeros(n, np.uint64)
    for b in range(10):
        for d in range(3):
            code |= ((q[:, d] >> np.uint64(b)) & np.uint64(1)) << np.uint64(3 * b + d)
    return np.argsort(code, kind="stable")


def _limbs(x, n):
    """Split f64 array into n bf16 limbs: x ~= sum(limbs)."""
    out = []
    r = x.copy()
    for _ in range(n):
        h = r.astype(NP_BF16)
        out.append(h)
        r = r - h.astype(np.float64)
    return out


def _host_prep(pos, coefficients, norm, center, alpha, lmn, orbital_index):
    """Returns (per_core in_maps, perm, profile (T0,T1 tuples), per-core
    window order arrays)."""
    pos = np.asarray(pos, np.float64)
    cn = (np.asarray(coefficients, np.float64) * np.asarray(norm, np.float64))
    center = np.asarray(center, np.float64)
    alpha = np.asarray(alpha, np.float64)
    lmn = np.asarray(lmn, np.int64)
    seg = np.asarray(orbital_index, np.int64)

    perm = _morton_perm(pos)
    spos = pos[perm]
    wpos = spos.reshape(N_CORES, N_WIN, WIN, 3)

    # ---- active prims per (core, window) ----
    sub = wpos[:, :, ::8, :]  # 64 sample points
    active = np.zeros((N_CORES, N_WIN, N_PRIM), bool)
    for k in range(N_CORES):
        for w in range(N_WIN):
            dx = sub[k, w][None, :, :] - center[:, None, :]
            r2 = (dx * dx).sum(-1)
            mono = (np.abs(dx) ** lmn[:, None, :]).prod(-1)
            v = np.abs(cn)[:, None] * mono * np.exp(-alpha[:, None] * r2)
            active[k, w] = v.max(1) > TOL

    in0 = seg < 128
    n0 = (active & in0[None, None, :]).sum(-1)
    n1 = (active & ~in0[None, None, :]).sum(-1)
    t0 = np.maximum(-(-n0 // PCH), 1)  # [K, W] ceil, min 1
    t1 = np.maximum(-(-n1 // PCH), 1)
    tot = t0 + t1
    order = np.argsort(-tot, axis=1, kind="stable")  # per-core rank -> window
    t0s = np.take_along_axis(t0, order, 1)
    t1s = np.take_along_axis(t1, order, 1)
    T0 = tuple(int(x) for x in t0s.max(0))
    T1 = tuple(int(x) for x in t1s.max(0))
    ch_of_slot = np.cumsum([0] + [T0[i] + T1[i] for i in range(N_WIN)])
    tot_ch = int(ch_of_slot[-1])

    ln2 = float(np.log(2.0))
    in_maps = []
    for k in range(N_CORES):
        cpos = spos[k * N_SH:(k + 1) * N_SH]
        blocks = cpos.reshape(N_WIN, WIN, 3)
        origins = blocks.mean(axis=1)
        dp0 = blocks - origins[:, None, :]
        lam = np.exp2(
            np.ceil(np.log2(np.maximum(np.abs(dp0).max(axis=(1, 2)), 1e-6) / 4.0))
        ).clip(min=1.0)
        dp = (dp0 / lam[:, None, None]).reshape(N_SH, 3)

        # A features (window-major), then permuted to slot order
        dpow = np.empty((3, 3, N_SH), np.float64)
        for d in range(3):
            dpow[d, 0] = 1.0
            dpow[d, 1] = dp[:, d]
            dpow[d, 2] = dp[:, d] ** 2
        a_mono = np.empty((27, N_SH), np.float64)
        for ki, (a, b, c) in enumerate(_EXPS):
            a_mono[ki] = dpow[0, a] * dpow[1, b] * dpow[2, c]
        r2p = dp[:, 0] ** 2 + dp[:, 1] ** 2 + dp[:, 2] ** 2
        a_expo = np.stack(
            [np.ones(N_SH), dp[:, 0], dp[:, 1], dp[:, 2], r2p], axis=0
        )
        am0, am1 = _limbs(a_mono, 2)
        at_m = np.zeros((KM, N_SH), NP_BF16)
        at_m[:81] = np.concatenate([am0, am1, am0], axis=0)
        ae0, ae1, ae2 = _limbs(a_expo, 3)
        at_e = np.zeros((KM, N_SH), NP_BF16)
        at_e[:30] = np.concatenate([ae0, ae1, ae2, ae0, ae1, ae0], axis=0)
        ord_k = order[k]
        at_m = at_m.reshape(KM, N_WIN, WIN)[:, ord_k, :].reshape(KM, N_SH)
        at_e = at_e.reshape(KM, N_WIN, WIN)[:, ord_k, :].reshape(KM, N_SH)

        # B tables per (window, prim), window-major
        cpr = center[None, :, :] - origins[:, None, :]  # [W, P, 3]
        npow = np.empty((N_WIN, N_PRIM, 3, 3), np.float64)
        npow[..., 0] = 1.0
        npow[..., 1] = -cpr
        npow[..., 2] = cpr ** 2
        bc = np.empty((N_WIN, N_PRIM, 3, 3), np.float64)
        for d in range(3):
            ld = lmn[:, d]
            for e in range(3):
                valid = (e <= ld)
                bcoef = _BINOM[ld, e]
                pw = npow[:, np.arange(N_PRIM), d, ld - e]
                bc[:, :, d, e] = np.where(valid[None, :], bcoef[None, :] * pw, 0.0)
        coefm = np.empty((N_WIN, N_PRIM, 27), np.float64)
        for ki, (a, b, c) in enumerate(_EXPS):
            coefm[:, :, ki] = (
                bc[:, :, 0, a] * bc[:, :, 1, b] * bc[:, :, 2, c]
                * (lam[:, None] ** (a + b + c))
            )
        coefm *= cn[None, :, None]
        maxc = np.abs(coefm).max(axis=2)
        s = np.ceil(np.log2(np.maximum(maxc, 1e-300) / 30000.0)).clip(min=0.0)
        coefm *= 2.0 ** (-s[:, :, None])

        c2 = (cpr ** 2).sum(axis=2)
        coefe = np.empty((N_WIN, N_PRIM, 5), np.float64)
        coefe[:, :, 0] = -alpha[None, :] * c2 + s * ln2
        for d in range(3):
            coefe[:, :, 1 + d] = 2.0 * alpha[None, :] * cpr[:, :, d] * lam[:, None]
        coefe[:, :, 4] = -alpha[None, :] * (lam ** 2)[:, None]

        bm0, bm1 = _limbs(coefm.transpose(0, 2, 1), 2)  # [W, 27, P]
        b_m = np.zeros((N_WIN, KM, N_PRIM), NP_BF16)
        b_m[:, :81] = np.concatenate([bm0, bm0, bm1], axis=1)
        be0, be1, be2 = _limbs(coefe.transpose(0, 2, 1), 3)
        b_e = np.zeros((N_WIN, KM, N_PRIM), NP_BF16)
        b_e[:, :30] = np.concatenate([be0, be0, be0, be1, be1, be2], axis=1)

        # ---- pack active prims into chunk slots ----
        # prims_of_chunk[ch, lane] = global prim id or -1
        prims = np.full((tot_ch, PCH), -1, np.int64)
        tile_of_chunk = np.zeros(tot_ch, np.int64)
        win_of_chunk = np.zeros(tot_ch, np.int64)
        for i in range(N_WIN):
            w = ord_k[i]
            act = active[k, w]
            p0 = np.nonzero(act & in0)[0]
            p1 = np.nonzero(act & ~in0)[0]
            base = ch_of_slot[i]
            for j in range(T0[i]):
                sel = p0[j * PCH:(j + 1) * PCH]
                prims[base + j, :len(sel)] = sel
                tile_of_chunk[base + j] = 0
                win_of_chunk[base + j] = w
            for j in range(T1[i]):
                ch = base + T0[i] + j
                sel = p1[j * PCH:(j + 1) * PCH]
                prims[ch, :len(sel)] = sel
                tile_of_chunk[ch] = 1
                win_of_chunk[ch] = w

        valid = prims >= 0
        pidx = np.where(valid, prims, 0)
        bm_pk = b_m[win_of_chunk[:, None], :, pidx]   # [CH, PCH, KM]
        be_pk = b_e[win_of_chunk[:, None], :, pidx]
        bm_pk[~valid] = 0
        be_pk[~valid] = 0
        bm_pk = np.ascontiguousarray(
            bm_pk.transpose(2, 0, 1).reshape(KM, tot_ch * PCH))
        be_pk = np.ascontiguousarray(
            be_pk.transpose(2, 0, 1).reshape(KM, tot_ch * PCH))

        s_pk = np.zeros((PCH, tot_ch * PCH), NP_BF16)
        ch_i, lane_i = np.nonzero(valid)
        orb = seg[prims[ch_i, lane_i]] - 128 * tile_of_chunk[ch_i]
        s_pk[lane_i, ch_i * PCH + orb] = 1.0

        in_maps.append(
            {
                "am": np.ascontiguousarray(at_m),
                "ae": np.ascontiguousarray(at_e),
                "bm": bm_pk,
                "be": be_pk,
                "s_pk": s_pk,
            }
        )
    return in_maps, perm, (T0, T1), order


def build_program(profile, n_sh=N_SH):
    T0, T1 = profile
    ch_of_slot = np.cumsum([0] + [T0[i] + T1[i] for i in range(N_WIN)])
    tot_ch = int(ch_of_slot[-1])
    nc = bacc.Bacc("TRN2", target_bir_lowering=False, debug=False,
                   num_devices=N_CORES)
    am_d = nc.dram_tensor("am", [KM, n_sh], BF16, kind="ExternalInput").ap()
    ae_d = nc.dram_tensor("ae", [KM, n_sh], BF16, kind="ExternalInput").ap()
    bm_d = nc.dram_tensor("bm", [KM, tot_ch * PCH], BF16, kind="ExternalInput").ap()
    be_d = nc.dram_tensor("be", [KM, tot_ch * PCH], BF16, kind="ExternalInput").ap()
    s_pk_d = nc.dram_tensor("s_pk", [PCH, tot_ch * PCH], BF16,
                            kind="ExternalInput").ap()
    out_d = nc.dram_tensor("out_t", [128, 2, n_sh], BF16, kind="ExternalOutput").ap()

    with tile.TileContext(nc) as tc:
        with (
            tc.tile_pool(name="cst", bufs=1) as cst,
            tc.tile_pool(name="wk", bufs=8) as wk,
            tc.tile_pool(name="ob", bufs=8) as ob,
            tc.tile_pool(name="pm", bufs=3, space="PSUM") as pm,
            tc.tile_pool(name="pex", bufs=3, space="PSUM") as pex,
            tc.tile_pool(name="po", bufs=1, space="PSUM") as po,
        ):
            am_t = cst.tile([KM, n_sh], BF16)
            ae_t = cst.tile([KM, n_sh], BF16)
            bm_t = cst.tile([KM, tot_ch * PCH], BF16)
            be_t = cst.tile([KM, tot_ch * PCH], BF16)
            s_t = cst.tile([PCH, tot_ch * PCH], BF16)
            # progressive splits: tiny first slice so slot-0 compute starts
            # ~10us earlier; later slices grow while compute overlaps
            for s0, s1 in ((0, 1), (1, 2), (2, 4), (4, 8), (8, N_WIN)):
                wsl = slice(s0 * WIN, s1 * WIN)
                c0 = int(ch_of_slot[s0]) * PCH
                c1 = int(ch_of_slot[s1]) * PCH
                nc.sync.dma_start(am_t[:, wsl], am_d[:, wsl])
                nc.sync.dma_start(ae_t[:, wsl], ae_d[:, wsl])
                nc.sync.dma_start(bm_t[:, c0:c1], bm_d[:, c0:c1])
                nc.sync.dma_start(be_t[:, c0:c1], be_d[:, c0:c1])
                nc.sync.dma_start(s_t[:, c0:c1], s_pk_d[:, c0:c1])

            for i in range(N_WIN):
                pot = po.tile([128, 2 * WIN], F32, tag="outp")
                psl = slice(i * WIN, (i + 1) * WIN)
                base = int(ch_of_slot[i])
                nch = T0[i] + T1[i]
                for j in range(nch):
                    ch = base + j
                    tl = 0 if j < T0[i] else 1
                    jj = j if tl == 0 else j - T0[i]
                    lastj = (T0[i] - 1) if tl == 0 else (nch - T0[i] - 1)
                    bsl = slice(ch * PCH, (ch + 1) * PCH)
                    mono_p = pm.tile([128, WIN], F32, tag="mono")
                    expo_p = pex.tile([128, WIN], F32, tag="expo")
                    nc.tensor.matmul(
                        mono_p[:], bm_t[:, bsl], am_t[:, psl],
                        start=True, stop=True,
                    )
                    nc.tensor.matmul(
                        expo_p[:], be_t[:, bsl], ae_t[:, psl],
                        start=True, stop=True,
                    )
                    e_t = wk.tile([128, WIN], BF16, tag="e")
                    nc.scalar.activation(e_t[:], expo_p[:], AF.Exp)
                    prim_t = wk.tile([128, WIN], BF16, tag="prim")
                    nc.vector.tensor_mul(prim_t[:], mono_p[:], e_t[:])
                    nc.tensor.matmul(
                        pot[:, tl * WIN:(tl + 1) * WIN],
                        s_t[:, bsl],
                        prim_t[:],
                        start=(jj == 0),
                        stop=(jj == lastj),
                    )
                osb = ob.tile([128, 2 * WIN], BF16, tag="osb")
                if i % 2 == 0:
                    nc.scalar.copy(osb[:], pot[:])
                else:
                    nc.vector.tensor_copy(osb[:], pot[:])
                for t in range(2):
                    nc.sync.dma_start(
                        out_d[:, t, psl], osb[:, t * WIN:(t + 1) * WIN]
                    )
    nc.compile()
    return nc


_PROG_CACHE = {}


def _get_program(profile):
    if profile not in _PROG_CACHE:
        _PROG_CACHE[profile] = build_program(profile)
    return _PROG_CACHE[profile]


def _install_ntff_hook_shim():
    """The agent image's antenv lacks axon_hooks; synthesize it so
    run_bass_kernel_spmd(trace=True) can capture NTFF profiles."""
    try:
        from antenv.axon_hooks import get_axon_ntff_profile_hook  # noqa: F401
        return True
    except ImportError:
        pass
    try:
        import types
        import antenv
        from trn_agent_boot.trn_boot import _ntff_profile_via_ctypes

        hook = _ntff_profile_via_ctypes("/opt/axon/libaxon_pjrt.so")
        mod = types.ModuleType("antenv.axon_hooks")
        mod._hook = hook
        mod.set_axon_ntff_profile_hook = lambda h: setattr(mod, "_hook", h)
        mod.get_axon_ntff_profile_hook = lambda: mod._hook
        sys.modules["antenv.axon_hooks"] = mod
        antenv.axon_hooks = mod
        return True
    except Exception as e:  # pragma: no cover
        print(f"ntff hook shim failed ({e}); running without trace")
        return False


def kernel(pos, coefficients, norm, center, alpha, lmn, orbital_index,
           num_orbitals):
    assert int(num_orbitals) == N_ORB and pos.shape == (N_POINTS, 3)
    in_maps, perm, profile, order = _host_prep(
        pos, coefficients, norm, center, alpha, lmn, orbital_index
    )
    nc = _get_program(profile)

    from concourse.bass_utils import run_bass_kernel_spmd

    trace = bool(os.environ.get("BASS_KERNEL_TRACE"))
    if trace:
        trace = _install_ntff_hook_shim()
    res = run_bass_kernel_spmd(nc, in_maps, list(range(N_CORES)), trace=trace)
    kernel.last_results = res

    full = np.empty((N_POINTS, N_ORB), np.float32)
    for k in range(N_CORES):
        v = res.results[k]["out_t"]  # [128, 2, N_SH] in slot order
        orb = v.transpose(1, 0, 2).reshape(N_ORB, N_WIN, WIN)
        orb = orb[:, np.argsort(order[k]), :].reshape(N_ORB, N_SH)
        full[k * N_SH:(k + 1) * N_SH] = orb.T.astype(np.float32)
    out = np.empty_like(full)
    out[perm] = full
    return out


# revision 18
# speedup vs baseline: 1.2230x; 1.0690x over previous
"""Trainium2 Bass kernel for nn_Basis (gaussian-basis orbital evaluation).

out[i, m] = sum_{p: orbital_index[p]==m} coeff[p]*norm[p]
            * prod_c (pos[i,c]-center[p,c])^lmn[p,c] * exp(-alpha[p]*|pos_i-center_p|^2)

Strategy (8 NeuronCores, data-parallel over points):
  - Host: Morton-sort points into 512-point windows with local origins.
    mono/expo expanded in dp = pos - o features as bf16-limb polynomials
    (81 / 30 K-rows, zero-padded to K=128: narrow-K matmuls hit a HW
    double-accumulate hazard; K=128 also enables FWL + 1 cyc/col).
  - Sparsity: per (core, window) only prims whose peak contribution over
    the window exceeds TOL are kept (~42%). Active prims are packed into
    128-wide chunk slots, split by orbital half (tile0: orb<128, tile1:
    orb>=128) so each chunk's segment matrix targets one PSUM half.
    A fixed per-rank capacity profile (max over cores) keeps the program
    identical across cores; windows are assigned to slots by rank.
  - Device per chunk slot:
      PE:  mono = Bmono^T @ A, expo = Bexpo^T @ Ae   (bf16, K=128)
      ACT: e = exp(expo) -> bf16
      DVE: prim = mono * e -> bf16
      PE:  pot[tile-half] += S_chunk^T @ prim   (0/1 bf16, PSUM accum)
  - Per-window [128, 2*512] PSUM out tile copied bf16 (ACT/DVE alternate)
    and DMA'd; host inverts slot assignment, Morton perm, orbital tiling.
"""
import os
import sys

sys.path.insert(0, "/opt/trn_rl_repo")

import numpy as np

import concourse.bass as bass
from concourse import bacc, mybir, tile
from concourse._compat import with_exitstack  # noqa: F401

import ml_dtypes

BF16 = mybir.dt.bfloat16
F32 = mybir.dt.float32
AF = mybir.ActivationFunctionType
NP_BF16 = ml_dtypes.bfloat16

N_POINTS = 65536
N_PRIM = 1024
N_ORB = 256
N_CORES = 8
N_SH = N_POINTS // N_CORES  # 8192 points per core
WIN = 512                   # points per window
PCH = 128                   # prims per chunk slot
N_WIN = N_SH // WIN         # 16 windows per core
KM = 128                    # K rows (81 mono / 30 expo used, rest zero)
TOL = 2e-3                  # abs prim-contribution cutoff

_EXPS = [(a, b, c) for a in range(3) for b in range(3) for c in range(3)]
_BINOM = np.array([[1, 0, 0], [1, 1, 0], [1, 2, 1]], dtype=np.float64)


def _morton_perm(pos):
    n = pos.shape[0]
    q = np.empty((n, 3), np.uint64)
    for d in range(3):
        x = pos[:, d].astype(np.float64)
        lo, hi = x.min(), x.max()
        q[:, d] = np.clip((x - lo) / max(hi - lo, 1e-9) * 1023.0, 0, 1023).astype(
            np.uint64
        )
    code = np.zeros(n, np.uint64)
    for b in range(10):
        for d in range(3):
            code |= ((q[:, d] >> np.uint64(b)) & np.uint64(1)) << np.uint64(3 * b + d)
    return np.argsort(code, kind="stable")


def _limbs(x, n):
    """Split f64 array into n bf16 limbs: x ~= sum(limbs)."""
    out = []
    r = x.copy()
    for _ in range(n):
        h = r.astype(NP_BF16)
        out.append(h)
        r = r - h.astype(np.float64)
    return out


def _host_prep(pos, coefficients, norm, center, alpha, lmn, orbital_index):
    """Returns (per_core in_maps, perm, profile (T0,T1 tuples), per-core
    window order arrays)."""
    pos = np.asarray(pos, np.float64)
    cn = (np.asarray(coefficients, np.float64) * np.asarray(norm, np.float64))
    center = np.asarray(center, np.float64)
    alpha = np.asarray(alpha, np.float64)
    lmn = np.asarray(lmn, np.int64)
    seg = np.asarray(orbital_index, np.int64)

    perm = _morton_perm(pos)
    spos = pos[perm]
    wpos = spos.reshape(N_CORES, N_WIN, WIN, 3)

    # ---- active prims per (core, window) ----
    sub = wpos[:, :, ::8, :]  # 64 sample points
    active = np.zeros((N_CORES, N_WIN, N_PRIM), bool)
    for k in range(N_CORES):
        for w in range(N_WIN):
            dx = sub[k, w][None, :, :] - center[:, None, :]
            r2 = (dx * dx).sum(-1)
            mono = (np.abs(dx) ** lmn[:, None, :]).prod(-1)
            v = np.abs(cn)[:, None] * mono * np.exp(-alpha[:, None] * r2)
            active[k, w] = v.max(1) > TOL

    in0 = seg < 128
    n0 = (active & in0[None, None, :]).sum(-1)
    n1 = (active & ~in0[None, None, :]).sum(-1)
    t0 = np.maximum(-(-n0 // PCH), 1)  # [K, W] ceil, min 1
    t1 = np.maximum(-(-n1 // PCH), 1)
    tot = t0 + t1
    order = np.argsort(-tot, axis=1, kind="stable")  # per-core rank -> window
    t0s = np.take_along_axis(t0, order, 1)
    t1s = np.take_along_axis(t1, order, 1)
    T0 = tuple(int(x) for x in t0s.max(0))
    T1 = tuple(int(x) for x in t1s.max(0))
    ch_of_slot = np.cumsum([0] + [T0[i] + T1[i] for i in range(N_WIN)])
    tot_ch = int(ch_of_slot[-1])

    ln2 = float(np.log(2.0))
    in_maps = []
    for k in range(N_CORES):
        cpos = spos[k * N_SH:(k + 1) * N_SH]
        blocks = cpos.reshape(N_WIN, WIN, 3)
        origins = blocks.mean(axis=1)
        dp0 = blocks - origins[:, None, :]
        lam = np.exp2(
            np.ceil(np.log2(np.maximum(np.abs(dp0).max(axis=(1, 2)), 1e-6) / 4.0))
        ).clip(min=1.0)
        dp = (dp0 / lam[:, None, None]).reshape(N_SH, 3)

        # A features (window-major), then permuted to slot order
        dpow = np.empty((3, 3, N_SH), np.float64)
        for d in range(3):
            dpow[d, 0] = 1.0
            dpow[d, 1] = dp[:, d]
            dpow[d, 2] = dp[:, d] ** 2
        a_mono = np.empty((27, N_SH), np.float64)
        for ki, (a, b, c) in enumerate(_EXPS):
            a_mono[ki] = dpow[0, a] * dpow[1, b] * dpow[2, c]
        r2p = dp[:, 0] ** 2 + dp[:, 1] ** 2 + dp[:, 2] ** 2
        a_expo = np.stack(
            [np.ones(N_SH), dp[:, 0], dp[:, 1], dp[:, 2], r2p], axis=0
        )
        am0, am1 = _limbs(a_mono, 2)
        at_m = np.zeros((KM, N_SH), NP_BF16)
        at_m[:81] = np.concatenate([am0, am1, am0], axis=0)
        ae0, ae1, ae2 = _limbs(a_expo, 3)
        at_e = np.zeros((KM, N_SH), NP_BF16)
        at_e[:30] = np.concatenate([ae0, ae1, ae2, ae0, ae1, ae0], axis=0)
        ord_k = order[k]
        at_m = at_m.reshape(KM, N_WIN, WIN)[:, ord_k, :].reshape(KM, N_SH)
        at_e = at_e.reshape(KM, N_WIN, WIN)[:, ord_k, :].reshape(KM, N_SH)

        # B tables per (window, prim), window-major
        cpr = center[None, :, :] - origins[:, None, :]  # [W, P, 3]
        npow = np.empty((N_WIN, N_PRIM, 3, 3), np.float64)
        npow[..., 0] = 1.0
        npow[..., 1] = -cpr
        npow[..., 2] = cpr ** 2
        bc = np.empty((N_WIN, N_PRIM, 3, 3), np.float64)
        for d in range(3):
            ld = lmn[:, d]
            for e in range(3):
                valid = (e <= ld)
                bcoef = _BINOM[ld, e]
                pw = npow[:, np.arange(N_PRIM), d, ld - e]
                bc[:, :, d, e] = np.where(valid[None, :], bcoef[None, :] * pw, 0.0)
        coefm = np.empty((N_WIN, N_PRIM, 27), np.float64)
        for ki, (a, b, c) in enumerate(_EXPS):
            coefm[:, :, ki] = (
                bc[:, :, 0, a] * bc[:, :, 1, b] * bc[:, :, 2, c]
                * (lam[:, None] ** (a + b + c))
            )
        coefm *= cn[None, :, None]
        maxc = np.abs(coefm).max(axis=2)
        s = np.ceil(np.log2(np.maximum(maxc, 1e-300) / 30000.0)).clip(min=0.0)
        coefm *= 2.0 ** (-s[:, :, None])

        c2 = (cpr ** 2).sum(axis=2)
        coefe = np.empty((N_WIN, N_PRIM, 5), np.float64)
        coefe[:, :, 0] = -alpha[None, :] * c2 + s * ln2
        for d in range(3):
            coefe[:, :, 1 + d] = 2.0 * alpha[None, :] * cpr[:, :, d] * lam[:, None]
        coefe[:, :, 4] = -alpha[None, :] * (lam ** 2)[:, None]

        bm0, bm1 = _limbs(coefm.transpose(0, 2, 1), 2)  # [W, 27, P]
        b_m = np.zeros((N_WIN, KM, N_PRIM), NP_BF16)
        b_m[:, :81] = np.concatenate([bm0, bm0, bm1], axis=1)
        be0, be1, be2 = _limbs(coefe.transpose(0, 2, 1), 3)
        b_e = np.zeros((N_WIN, KM, N_PRIM), NP_BF16)
        b_e[:, :30] = np.concatenate([be0, be0, be0, be1, be1, be2], axis=1)

        # ---- pack active prims into chunk slots ----
        # prims_of_chunk[ch, lane] = global prim id or -1
        prims = np.full((tot_ch, PCH), -1, np.int64)
        tile_of_chunk = np.zeros(tot_ch, np.int64)
        win_of_chunk = np.zeros(tot_ch, np.int64)
        for i in range(N_WIN):
            w = ord_k[i]
            act = active[k, w]
            p0 = np.nonzero(act & in0)[0]
            p1 = np.nonzero(act & ~in0)[0]
            base = ch_of_slot[i]
            for j in range(T0[i]):
                sel = p0[j * PCH:(j + 1) * PCH]
                prims[base + j, :len(sel)] = sel
                tile_of_chunk[base + j] = 0
                win_of_chunk[base + j] = w
            for j in range(T1[i]):
                ch = base + T0[i] + j
                sel = p1[j * PCH:(j + 1) * PCH]
                prims[ch, :len(sel)] = sel
                tile_of_chunk[ch] = 1
                win_of_chunk[ch] = w

        valid = prims >= 0
        pidx = np.where(valid, prims, 0)
        bm_pk = b_m[win_of_chunk[:, None], :, pidx]   # [CH, PCH, KM]
        be_pk = b_e[win_of_chunk[:, None], :, pidx]
        bm_pk[~valid] = 0
        be_pk[~valid] = 0
        bm_pk = np.ascontiguousarray(
            bm_pk.transpose(2, 0, 1).reshape(KM, tot_ch * PCH))
        be_pk = np.ascontiguousarray(
            be_pk.transpose(2, 0, 1).reshape(KM, tot_ch * PCH))

        s_pk = np.zeros((PCH, tot_ch * PCH), NP_BF16)
        ch_i, lane_i = np.nonzero(valid)
        orb = seg[prims[ch_i, lane_i]] - 128 * tile_of_chunk[ch_i]
        s_pk[lane_i, ch_i * PCH + orb] = 1.0

        in_maps.append(
            {
                "am": np.ascontiguousarray(at_m),
                "ae": np.ascontiguousarray(at_e),
                "bm": bm_pk,
                "be": be_pk,
                "s_pk": s_pk,
            }
        )
    return in_maps, perm, (T0, T1), order


def build_program(profile, n_sh=N_SH):
    T0, T1 = profile
    ch_of_slot = np.cumsum([0] + [T0[i] + T1[i] for i in range(N_WIN)])
    tot_ch = int(ch_of_slot[-1])
    nc = bacc.Bacc("TRN2", target_bir_lowering=False, debug=False,
                   num_devices=N_CORES)
    am_d = nc.dram_tensor("am", [KM, n_sh], BF16, kind="ExternalInput").ap()
    ae_d = nc.dram_tensor("ae", [KM, n_sh], BF16, kind="ExternalInput").ap()
    bm_d = nc.dram_tensor("bm", [KM, tot_ch * PCH], BF16, kind="ExternalInput").ap()
    be_d = nc.dram_tensor("be", [KM, tot_ch * PCH], BF16, kind="ExternalInput").ap()
    s_pk_d = nc.dram_tensor("s_pk", [PCH, tot_ch * PCH], BF16,
                            kind="ExternalInput").ap()
    out_d = nc.dram_tensor("out_t", [128, 2, n_sh], BF16, kind="ExternalOutput").ap()

    with tile.TileContext(nc) as tc:
        with (
            tc.tile_pool(name="cst", bufs=1) as cst,
            tc.tile_pool(name="wk", bufs=8) as wk,
            tc.tile_pool(name="ob", bufs=8) as ob,
            tc.tile_pool(name="pm", bufs=3, space="PSUM") as pm,
            tc.tile_pool(name="pex", bufs=3, space="PSUM") as pex,
            tc.tile_pool(name="po", bufs=1, space="PSUM") as po,
        ):
            am_t = cst.tile([KM, n_sh], BF16)
            ae_t = cst.tile([KM, n_sh], BF16)
            bm_t = cst.tile([KM, tot_ch * PCH], BF16)
            be_t = cst.tile([KM, tot_ch * PCH], BF16)
            s_t = cst.tile([PCH, tot_ch * PCH], BF16)
            # progressive splits: tiny first slice so slot-0 compute starts
            # ~10us earlier; later slices grow while compute overlaps
            for s0, s1 in ((0, 1), (1, 2), (2, 4), (4, 8), (8, N_WIN)):
                wsl = slice(s0 * WIN, s1 * WIN)
                c0 = int(ch_of_slot[s0]) * PCH
                c1 = int(ch_of_slot[s1]) * PCH
                nc.sync.dma_start(am_t[:, wsl], am_d[:, wsl])
                nc.sync.dma_start(ae_t[:, wsl], ae_d[:, wsl])
                nc.sync.dma_start(bm_t[:, c0:c1], bm_d[:, c0:c1])
                nc.sync.dma_start(be_t[:, c0:c1], be_d[:, c0:c1])
                nc.sync.dma_start(s_t[:, c0:c1], s_pk_d[:, c0:c1])

            for i in range(N_WIN):
                pot = po.tile([128, 2 * WIN], F32, tag="outp")
                psl = slice(i * WIN, (i + 1) * WIN)
                base = int(ch_of_slot[i])
                nch = T0[i] + T1[i]
                for j in range(nch):
                    ch = base + j
                    tl = 0 if j < T0[i] else 1
                    jj = j if tl == 0 else j - T0[i]
                    lastj = (T0[i] - 1) if tl == 0 else (nch - T0[i] - 1)
                    bsl = slice(ch * PCH, (ch + 1) * PCH)
                    mono_p = pm.tile([128, WIN], F32, tag="mono")
                    expo_p = pex.tile([128, WIN], F32, tag="expo")
                    nc.tensor.matmul(
                        mono_p[:], bm_t[:, bsl], am_t[:, psl],
                        start=True, stop=True,
                    )
                    nc.tensor.matmul(
                        expo_p[:], be_t[:, bsl], ae_t[:, psl],
                        start=True, stop=True,
                    )
                    e_t = wk.tile([128, WIN], BF16, tag="e")
                    nc.scalar.activation(e_t[:], expo_p[:], AF.Exp)
                    prim_t = wk.tile([128, WIN], BF16, tag="prim")
                    nc.vector.tensor_mul(prim_t[:], mono_p[:], e_t[:])
                    nc.tensor.matmul(
                        pot[:, tl * WIN:(tl + 1) * WIN],
                        s_t[:, bsl],
                        prim_t[:],
                        start=(jj == 0),
                        stop=(jj == lastj),
                    )
                osb = ob.tile([128, 2 * WIN], BF16, tag="osb")
                if i % 2 == 0:
                    nc.scalar.copy(osb[:], pot[:])
                else:
                    nc.vector.tensor_copy(osb[:], pot[:])
                for t in range(2):
                    nc.sync.dma_start(
                        out_d[:, t, psl], osb[:, t * WIN:(t + 1) * WIN]
                    )
    nc.compile()
    return nc


_PROG_CACHE = {}


def _get_program(profile):
    if profile not in _PROG_CACHE:
        _PROG_CACHE[profile] = build_program(profile)
    return _PROG_CACHE[profile]


def _install_ntff_hook_shim():
    """The agent image's antenv lacks axon_hooks; synthesize it so
    run_bass_kernel_spmd(trace=True) can capture NTFF profiles."""
    try:
        from antenv.axon_hooks import get_axon_ntff_profile_hook  # noqa: F401
        return True
    except ImportError:
        pass
    try:
        import types
        import antenv
        from trn_agent_boot.trn_boot import _ntff_profile_via_ctypes

        hook = _ntff_profile_via_ctypes("/opt/axon/libaxon_pjrt.so")
        mod = types.ModuleType("antenv.axon_hooks")
        mod._hook = hook
        mod.set_axon_ntff_profile_hook = lambda h: setattr(mod, "_hook", h)
        mod.get_axon_ntff_profile_hook = lambda: mod._hook
        sys.modules["antenv.axon_hooks"] = mod
        antenv.axon_hooks = mod
        return True
    except Exception as e:  # pragma: no cover
        print(f"ntff hook shim failed ({e}); running without trace")
        return False


def kernel(pos, coefficients, norm, center, alpha, lmn, orbital_index,
           num_orbitals):
    assert int(num_orbitals) == N_ORB and pos.shape == (N_POINTS, 3)
    in_maps, perm, profile, order = _host_prep(
        pos, coefficients, norm, center, alpha, lmn, orbital_index
    )
    nc = _get_program(profile)

    from concourse.bass_utils import run_bass_kernel_spmd

    trace = bool(os.environ.get("BASS_KERNEL_TRACE"))
    if trace:
        trace = _install_ntff_hook_shim()
    res = run_bass_kernel_spmd(nc, in_maps, list(range(N_CORES)), trace=trace)
    kernel.last_results = res

    full = np.empty((N_POINTS, N_ORB), np.float32)
    for k in range(N_CORES):
        v = res.results[k]["out_t"]  # [128, 2, N_SH] in slot order
        orb = v.transpose(1, 0, 2).reshape(N_ORB, N_WIN, WIN)
        orb = orb[:, np.argsort(order[k]), :].reshape(N_ORB, N_SH)
        full[k * N_SH:(k + 1) * N_SH] = orb.T.astype(np.float32)
    out = np.empty_like(full)
    out[perm] = full
    return out
